# revision 1
# baseline (speedup 1.0000x reference)
"""Cross-attention kernel for Trainium2, 8 NeuronCores, data-parallel over batch.

Problem (per batch element b, one per core):
    q  = x_b @ Wq.T + bq                      [T=1024, C=1024]
    kv = enc_b @ Wkv.T + bkv                  [I=576, 2C]
    per head h (H=16, D=64):
        att = softmax((q_h @ k_h.T) / sqrt(D))
        y_h = att @ v_h
    out = y @ Wo.T + bo                       [T, C]

Design notes:
  - One batch element per core (B=8 == n_cores), no collectives.
  - Weights are pre-transposed on host to [in, out] layout so the
    contraction dim (c) lands on SBUF partitions for matmuls.
  - x / enc are transposed on-device via the PE (out = in.T @ I).
  - Matmuls run as float32r (TF32-like, 1 cyc/row at N>=256) via AP bitcast.
  - Attention is computed in S^T = K_h @ Q_h^T orientation ([i, t]); exp is
    applied without max-subtraction (scores are O(1), exp <= ~e^6).  The
    softmax denominator Z_t falls out of the AV matmul by augmenting V with
    a ones column (lhsT M=65); normalization multiplies y^T by a rank-1
    PE-broadcast of 1/Z.
  - Biases: bq/bk are per-partition adds; bv/bo are rank-1 (K=1) matmul
    accumulates of ones^T (x) bias_row.
"""

import numpy as np

T = 1024
C = 1024
I = 576
H = 16
D = 64
NCC = C // 128          # 8 contraction chunks
NIC = (I + 127) // 128  # 5 i chunks (128,128,128,128,64)
I_CH = [128, 128, 128, 128, 64]
VW = 68                 # per-head column block in V tile: 64 v cols + ones col + pad
SCALE = 1.0 / np.sqrt(D)

_CACHE = {}


def _build_nc():
    import concourse.bass as bass
    import concourse.bacc as bacc
    import concourse.mybir as mybir
    import concourse.tile as tile
    from contextlib import ExitStack

    f32 = mybir.dt.float32
    f32r = mybir.dt.float32r

    nc = bacc.Bacc()

    x_d = nc.dram_tensor("x", [T, C], f32r, kind="ExternalInput")
    enc_d = nc.dram_tensor("enc", [I, C], f32r, kind="ExternalInput")
    wqT_d = nc.dram_tensor("wqT", [C, C], f32r, kind="ExternalInput")
    wkT_d = nc.dram_tensor("wkT", [C, C], f32r, kind="ExternalInput")
    wvT_d = nc.dram_tensor("wvT", [C, C], f32r, kind="ExternalInput")
    woT_d = nc.dram_tensor("woT", [C, C], f32r, kind="ExternalInput")
    bq_d = nc.dram_tensor("bq", [C], f32, kind="ExternalInput")
    bk_d = nc.dram_tensor("bk", [C], f32, kind="ExternalInput")
    bv_d = nc.dram_tensor("bv", [C], f32r, kind="ExternalInput")
    bo_d = nc.dram_tensor("bo", [C], f32r, kind="ExternalInput")
    out_d = nc.dram_tensor("out", [T, C], f32, kind="ExternalOutput")

    with ExitStack() as ctx:
        tc = ctx.enter_context(tile.TileContext(nc))

        # long-lived pools
        resid = ctx.enter_context(tc.tile_pool(name="resid", bufs=1))
        misc = ctx.enter_context(tc.tile_pool(name="misc", bufs=1))
        pa = ctx.enter_context(tc.tile_pool(name="pa", bufs=6, space="PSUM"))
        exps = ctx.enter_context(tc.tile_pool(name="exps", bufs=10))

        # constants (DMA'd from NEFF-embedded data; engines can't memset f32r)
        ident_d = nc.inline_tensor(np.eye(128, dtype=np.float32), name="ident_d")
        ones_d = nc.inline_tensor(np.ones((128, 128), dtype=np.float32), name="ones_d")
        ident = misc.tile([128, 128], f32r)
        nc.sync.dma_start(out=ident, in_=ident_d[:, :].bitcast(f32r))
        ones_t = misc.tile([128, 128], f32r)
        nc.sync.dma_start(out=ones_t, in_=ones_d[:, :].bitcast(f32r))
        bq_t = misc.tile([128, NCC], f32)
        nc.sync.dma_start(out=bq_t, in_=bq_d[:].rearrange("(oc p) -> p oc", p=128))
        bk_t = misc.tile([128, NCC], f32)
        nc.sync.dma_start(out=bk_t, in_=bk_d[:].rearrange("(oc p) -> p oc", p=128))
        bv_row = misc.tile([1, C], f32r)
        nc.sync.dma_start(out=bv_row, in_=bv_d[:].unsqueeze(0))
        bo_row = misc.tile([1, C], f32r)
        nc.sync.dma_start(out=bo_row, in_=bo_d[:].unsqueeze(0))

        # resident tensors
        QT = [resid.tile([128, T], f32r, tag=f"QT{i}", name=f"QT{i}") for i in range(NCC)]
        KT = [resid.tile([128, I], f32r, tag=f"KT{i}", name=f"KT{i}") for i in range(NCC)]
        V3 = [resid.tile([128, H, VW], f32r, tag=f"V{i}", name=f"V{i}") for i in range(NIC)]
        YT = [resid.tile([128, T], f32r, tag=f"YT{i}", name=f"YT{i}") for i in range(NCC)]

        with tc.tile_pool(name="ph1", bufs=1) as ph1, \
             tc.tile_pool(name="xin", bufs=3) as xin, \
             tc.tile_pool(name="wsm", bufs=4) as wsm, \
             tc.tile_pool(name="wv8", bufs=1) as wv8, \
             tc.tile_pool(name="pt", bufs=2, space="PSUM") as pt:

            # ---- enc^T (resident through V proj) ----
            encT = [ph1.tile([128, I], f32r, tag=f"encT{i}", name=f"encT{i}") for i in range(NCC)]
            for ii in range(NIC):
                pi = I_CH[ii]
                e_nat = xin.tile([128, C], f32r, tag="xin")
                nc.sync.dma_start(out=e_nat[:pi], in_=enc_d[ii * 128 : ii * 128 + pi])
                for cc in range(NCC):
                    ps = pt.tile([128, 128], f32r, tag="pt")
                    nc.tensor.transpose(
                        ps[:128, :pi],
                        e_nat[:pi, cc * 128 : (cc + 1) * 128],
                        ident[:pi, :pi],
                    )
                    nc.vector.tensor_copy(
                        encT[cc][:, ii * 128 : ii * 128 + pi], ps[:128, :pi]
                    )

            # ---- x^T in t-halves + Q^T projection ----
            for tch in range(2):
                xTh = [ph1.tile([128, 512], f32r, tag=f"xTh{i}", name=f"xTh{i}") for i in range(NCC)]
                for ts in range(4):
                    tt = tch * 4 + ts
                    x_nat = xin.tile([128, C], f32r, tag="xin")
                    nc.sync.dma_start(out=x_nat, in_=x_d[tt * 128 : (tt + 1) * 128])
                    for cc in range(NCC):
                        ps = pt.tile([128, 128], f32r, tag="pt")
                        nc.tensor.transpose(
                            ps, x_nat[:, cc * 128 : (cc + 1) * 128], ident
                        )
                        nc.vector.tensor_copy(
                            xTh[cc][:, ts * 128 : (ts + 1) * 128], ps
                        )
                # Q^T[o, t-half] = (WqT).T @ x^T ; accumulate over c chunks
                for oc in range(NCC):
                    pq = pa.tile([128, 512], f32, tag="pa")
                    for cc in range(NCC):
                        wch = wsm.tile([128, 128], f32r, tag="wsm")
                        nc.sync.dma_start(
                            out=wch,
                            in_=wqT_d[
                                cc * 128 : (cc + 1) * 128, oc * 128 : (oc + 1) * 128
                            ],
                        )
                        nc.tensor.matmul(
                            pq,
                            wch,
                            xTh[cc],
                            start=(cc == 0),
                            stop=(cc == NCC - 1),
                        )
                    nc.vector.tensor_scalar_add(
                        QT[oc][:, tch * 512 : (tch + 1) * 512],
                        pq,
                        bq_t[:, oc : oc + 1],
                    )

            # ---- K^T projection (i in halves of 288) ----
            for oc in range(NCC):
                pk = [pa.tile([128, 288], f32, tag="pa", name=f"pk{_}") for _ in range(2)]
                for cc in range(NCC):
                    wch = wsm.tile([128, 128], f32r, tag="wsm")
                    nc.sync.dma_start(
                        out=wch,
                        in_=wkT_d[
                            cc * 128 : (cc + 1) * 128, oc * 128 : (oc + 1) * 128
                        ],
                    )
                    for ih in range(2):
                        nc.tensor.matmul(
                            pk[ih],
                            wch,
                            encT[cc][:, ih * 288 : (ih + 1) * 288],
                            start=(cc == 0),
                            stop=(cc == NCC - 1),
                        )
                for ih in range(2):
                    nc.vector.tensor_scalar_add(
                        KT[oc][:, ih * 288 : (ih + 1) * 288],
                        pk[ih],
                        bk_t[:, oc : oc + 1],
                    )

            # ---- V projection into [128, H, VW] layout with ones columns ----
            for ii in range(NIC):
                # ones column (head-block col 64) for the fused Z row in AV
                nc.sync.dma_start(
                    out=V3[ii][:, :, 64:65],
                    in_=ones_d[:, 0:H].bitcast(f32r).unsqueeze(2),
                )
            for och in range(2):
                wvt = [wv8.tile([128, 512], f32r, tag=f"wv{i}", name=f"wv{i}") for i in range(NCC)]
                for cc in range(NCC):
                    nc.sync.dma_start(
                        out=wvt[cc],
                        in_=wvT_d[cc * 128 : (cc + 1) * 128, och * 512 : (och + 1) * 512],
                    )
                for ii in range(NIC):
                    pi = I_CH[ii]
                    pv = pa.tile([128, 512], f32, tag="pa")
                    for cc in range(NCC):
                        nc.tensor.matmul(
                            pv[:pi],
                            encT[cc][:, ii * 128 : ii * 128 + pi],
                            wvt[cc],
                            start=(cc == 0),
                            stop=False,
                        )
                    # bv: rank-1 ones^T (x) bv_row accumulate
                    nc.tensor.matmul(
                        pv[:pi],
                        ones_t[0:1, :pi],
                        bv_row[0:1, och * 512 : (och + 1) * 512],
                        start=False,
                        stop=True,
                    )
                    dst = V3[ii][:pi, och * 8 : och * 8 + 8, 0:64]
                    nc.vector.tensor_copy(
                        dst, pv[:pi].rearrange("p (h d) -> p h d", d=64)
                    )

        # ---- attention ----
        with tc.tile_pool(name="attn", bufs=3) as attn:
            for h in range(H):
                oc = h // 2
                hb = (h % 2) * 64
                for tch in range(2):
                    tsl = slice(tch * 512, (tch + 1) * 512)
                    # S^T chunks -> exp -> sbuf
                    es = []
                    for ii in range(NIC):
                        pi = I_CH[ii]
                        ps = pa.tile([128, 512], f32, tag="pa")
                        nc.tensor.matmul(
                            ps[:pi],
                            KT[oc][hb : hb + 64, ii * 128 : ii * 128 + pi],
                            QT[oc][hb : hb + 64, tsl],
                            start=True,
                            stop=True,
                        )
                        e = exps.tile([128, 512], f32r, tag="exps")
                        nc.scalar.activation(
                            e[:pi],
                            ps[:pi],
                            mybir.ActivationFunctionType.Exp,
                            scale=float(SCALE),
                        )
                        es.append(e)
                    # y^T (64 rows) and Z (row 64) via V augmented with ones col
                    py = pa.tile([128, 512], f32, tag="pa")
                    for ii in range(NIC):
                        pi = I_CH[ii]
                        nc.tensor.matmul(
                            py[:65],
                            V3[ii][:pi, h, 0:65],
                            es[ii][:pi],
                            start=(ii == 0),
                            stop=(ii == NIC - 1),
                        )
                    # r = 1/Z on partition 64; rank-1 broadcast to [64, 512]
                    rz = attn.tile([128, 512], f32r, tag="rz")
                    with nc.allow_low_precision(reason="1/Z in f32r is fine"):
                        nc.vector.reciprocal(rz[64:65], py[64:65])
                    pb = pa.tile([128, 512], f32, tag="pa")
                    nc.tensor.matmul(
                        pb[:64],
                        ones_t[64:65, 0:64],
                        rz[64:65],
                        start=True,
                        stop=True,
                    )
                    zb = attn.tile([64, 512], f32, tag="zb")
                    nc.vector.tensor_copy(zb, pb[:64])
                    nc.vector.tensor_mul(YT[oc][hb : hb + 64, tsl], py[:64], zb)

        # ---- output projection ----
        with tc.tile_pool(name="wo16", bufs=1) as wo16, \
             tc.tile_pool(name="osb", bufs=3) as osb:
            wot = {}
            for cc in range(NCC):
                for och in range(2):
                    w = wo16.tile([128, 512], f32r, tag=f"wo{cc}_{och}", name=f"wo{cc}_{och}")
                    nc.sync.dma_start(
                        out=w,
                        in_=woT_d[
                            cc * 128 : (cc + 1) * 128, och * 512 : (och + 1) * 512
                        ],
                    )
                    wot[(cc, och)] = w
            for tt in range(8):
                ot = osb.tile([128, C], f32, tag="osb")
                for och in range(2):
                    po = pa.tile([128, 512], f32, tag="pa")
                    for cc in range(NCC):
                        nc.tensor.matmul(
                            po,
                            YT[cc][:, tt * 128 : (tt + 1) * 128],
                            wot[(cc, och)],
                            start=(cc == 0),
                            stop=False,
                        )
                    nc.tensor.matmul(
                        po,
                        ones_t[0:1, 0:128],
                        bo_row[0:1, och * 512 : (och + 1) * 512],
                        start=False,
                        stop=True,
                    )
                    nc.vector.tensor_copy(ot[:, och * 512 : (och + 1) * 512], po)
                nc.sync.dma_start(out=out_d[tt * 128 : (tt + 1) * 128], in_=ot)

    nc.compile()
    return nc


def _get_nc():
    if "nc" not in _CACHE:
        _CACHE["nc"] = _build_nc()
    return _CACHE["nc"]


def _prep_in_maps(x, encoder_output, Wq, bq, Wkv, bkv, Wo, bo):
    f = np.float32
    x = np.asarray(x, f)
    enc = np.asarray(encoder_output, f)
    wqT = np.ascontiguousarray(np.asarray(Wq, f).T)
    wkv = np.asarray(Wkv, f)
    wkT = np.ascontiguousarray(wkv[:C].T)
    wvT = np.ascontiguousarray(wkv[C:].T)
    woT = np.ascontiguousarray(np.asarray(Wo, f).T)
    bq = np.asarray(bq, f)
    bkv = np.asarray(bkv, f)
    bo = np.asarray(bo, f)
    shared = {
        "wqT": wqT, "wkT": wkT, "wvT": wvT, "woT": woT,
        "bq": bq, "bk": np.ascontiguousarray(bkv[:C]),
        "bv": np.ascontiguousarray(bkv[C:]), "bo": bo,
    }
    return [
        dict(shared, x=np.ascontiguousarray(x[b]), enc=np.ascontiguousarray(enc[b]))
        for b in range(x.shape[0])
    ]


def kernel(x, encoder_output, Wq, bq, Wkv, bkv, Wo, bo):
    from concourse.bass_utils import run_bass_kernel_spmd

    nc = _get_nc()
    in_maps = _prep_in_maps(x, encoder_output, Wq, bq, Wkv, bkv, Wo, bo)
    res = run_bass_kernel_spmd(nc, in_maps, list(range(len(in_maps)))).results
    return np.stack([res[b]["out"] for b in range(len(res))]).astype(np.float32)



# revision 10
# speedup vs baseline: 2.2350x; 2.2350x over previous
"""Cross-attention kernel for Trainium2, 8 NeuronCores, data-parallel over batch.

Problem (per batch element b, one per core):
    q  = x_b @ Wq.T + bq                      [T=1024, C=1024]
    kv = enc_b @ Wkv.T + bkv                  [I=576, 2C]
    per head h (H=16, D=64):
        att = softmax((q_h @ k_h.T) / sqrt(D))
        y_h = att @ v_h
    out = y @ Wo.T + bo                       [T, C]

Design notes (v2):
  - One batch element per core (B=8 == n_cores), no collectives.
  - All transposes are done on HOST: x^T / enc^T / W^T arrive pre-laid-out
    as [128, 8, N] bf16 so the contraction dim (c) is on SBUF partitions.
    No PE transposes, no PSUM round-trips for layout.
  - All matmul operands are bf16 (cast on host); PSUM accumulation stays
    f32.  End-to-end rel err ~7e-3 (vs 2e-2 tolerance).
  - Each weight is DMA'd exactly once with 16KB-contiguous partition lines.
  - Attention: S^T = K_h @ Q_h^T per head in [i, t] orientation; exp without
    max-subtraction into bf16; one ACT instruction per [i-chunk, 1024] (both
    t-halves).  The softmax denominator Z falls out of the AV matmul via a
    ones column in V (lhsT M=65).  1/Z via reciprocal_approx_fast straight
    off the PSUM Z row into a resident [16, T] table; normalization is a
    rank-2 PE broadcast (2 heads at once) + one DVE multiply, deferred a
    few heads to stay off the critical path (keeps HAM un-throttled).
  - Biases: bq/bk are per-partition adds; bv/bo are rank-1 (K=1) matmul
    accumulates of ones^T (x) bias_row.
"""

import numpy as np
import ml_dtypes

T = 1024
C = 1024
I = 576
H = 16
D = 64
NCC = C // 128          # 8 contraction chunks
NIC = (I + 127) // 128  # 5 i chunks (128,128,128,128,64)
I_CH = [128, 128, 128, 128, 64]
VW = 68                 # per-head column block in V tile: 64 v cols + ones col + pad
SCALE = 1.0 / np.sqrt(D)

_CACHE = {}


def _build_nc():
    import concourse.bass as bass
    import concourse.bacc as bacc
    import concourse.mybir as mybir
    import concourse.tile as tile
    from concourse.dve_ops import RECIP_APPROX_FAST_CONSTS, RECIPROCAL_APPROX_FAST
    from contextlib import ExitStack

    f32 = mybir.dt.float32
    f32r = mybir.dt.float32r
    bf16 = mybir.dt.bfloat16

    nc = bacc.Bacc()

    # host-pre-transposed inputs: [p, cc, n] with c = cc*128 + p on partitions
    xt_d = nc.dram_tensor("xt", [128, NCC, T], bf16, kind="ExternalInput")
    ect_d = nc.dram_tensor("ect", [128, NCC, I], bf16, kind="ExternalInput")
    wq_d = nc.dram_tensor("wq", [128, NCC, C], bf16, kind="ExternalInput")
    wk_d = nc.dram_tensor("wk", [128, NCC, C], bf16, kind="ExternalInput")
    wv_d = nc.dram_tensor("wv", [128, NCC, C], bf16, kind="ExternalInput")
    wo_d = nc.dram_tensor("wo", [128, NCC, C], bf16, kind="ExternalInput")
    bq_d = nc.dram_tensor("bqp", [128, NCC], f32, kind="ExternalInput")
    bk_d = nc.dram_tensor("bkp", [128, NCC], f32, kind="ExternalInput")
    bv_d = nc.dram_tensor("bv", [C], bf16, kind="ExternalInput")
    bo_d = nc.dram_tensor("bo", [C], bf16, kind="ExternalInput")
    out_d = nc.dram_tensor("out", [T, C], f32, kind="ExternalOutput")

    # consts
    vones_d = nc.inline_tensor(np.ones((128, 16), dtype=ml_dtypes.bfloat16), name="vones_d")
    onesr_d = nc.inline_tensor(np.ones((1, 128), dtype=ml_dtypes.bfloat16), name="onesr_d")
    # sel4c[z, j, m]: picks Z rows {0,32} (j=0) or {64,96} (j=1) into row-halves
    sel4_np = np.zeros((128, 2, 128), dtype=np.float32)
    sel4_np[0, 0, 0:64] = 1.0
    sel4_np[32, 0, 64:128] = 1.0
    sel4_np[64, 1, 0:64] = 1.0
    sel4_np[96, 1, 64:128] = 1.0
    sel4_d = nc.inline_tensor(sel4_np, name="sel4_d")

    with ExitStack() as ctx:
        tc = ctx.enter_context(tile.TileContext(nc))

        resid = ctx.enter_context(tc.tile_pool(name="resid", bufs=1))
        misc = ctx.enter_context(tc.tile_pool(name="misc", bufs=1))
        exps = ctx.enter_context(tc.tile_pool(name="exps", bufs=8))

        # consts + biases
        ones_r = misc.tile([1, 128], bf16)
        nc.sync.dma_start(out=ones_r, in_=onesr_d[:, :])
        sel4 = misc.tile([128, 2, 128], f32r)
        nc.sync.dma_start(out=sel4, in_=sel4_d[:, :, :].bitcast(f32r))
        bq_t = misc.tile([128, NCC], f32)
        nc.sync.dma_start(out=bq_t, in_=bq_d[:, :])
        bk_t = misc.tile([128, NCC], f32)
        nc.sync.dma_start(out=bk_t, in_=bk_d[:, :])
        bv_row = misc.tile([1, C], bf16)
        nc.sync.dma_start(out=bv_row, in_=bv_d[:].unsqueeze(0))
        bo_row = misc.tile([1, C], bf16)
        nc.sync.dma_start(out=bo_row, in_=bo_d[:].unsqueeze(0))

        # resident tensors
        QT = [resid.tile([128, T], bf16, tag=f"QT{i}", name=f"QT{i}") for i in range(NCC)]
        KT = [resid.tile([128, I], bf16, tag=f"KT{i}", name=f"KT{i}") for i in range(NCC)]
        V3 = [resid.tile([128, H, VW], bf16, tag=f"V{i}", name=f"V{i}") for i in range(NIC)]
        YTu = [resid.tile([128, T], bf16, tag=f"YTu{i}", name=f"YTu{i}") for i in range(NCC)]
        YT = [resid.tile([128, T], bf16, tag=f"YT{i}", name=f"YT{i}") for i in range(NCC)]
        Zc = [resid.tile([128, 512], f32, tag=f"Zc{i}", name=f"Zc{i}") for i in range(8)]
        Zi = [resid.tile([128, 512], f32r, tag=f"Zi{i}", name=f"Zi{i}") for i in range(8)]

        with tc.tile_pool(name="ph1", bufs=1) as ph1, \
             tc.tile_pool(name="pp", bufs=6, space="PSUM") as pp:

            # stage all big inputs once, in first-use order
            xt = ph1.tile([128, NCC, T], bf16, tag="xt", name="xt")
            nc.sync.dma_start(out=xt, in_=xt_d[:, :, :])
            wqt = ph1.tile([128, NCC, C], bf16, tag="wqt", name="wqt")
            nc.sync.dma_start(out=wqt, in_=wq_d[:, :, :])
            ect = ph1.tile([128, NCC, I], bf16, tag="ect", name="ect")
            nc.sync.dma_start(out=ect, in_=ect_d[:, :, :])
            wkt = ph1.tile([128, NCC, C], bf16, tag="wkt", name="wkt")
            nc.sync.dma_start(out=wkt, in_=wk_d[:, :, :])
            wvt = ph1.tile([128, NCC, C], bf16, tag="wvt", name="wvt")
            nc.sync.dma_start(out=wvt, in_=wv_d[:, :, :])
            # ones column (head-block col 64) for the fused Z row in AV
            for ii in range(NIC):
                nc.sync.dma_start(
                    out=V3[ii][:, :, 64:65], in_=vones_d[:, 0:H].unsqueeze(2)
                )

            # ---- Q^T projection: QT[oc][o_p, t] ----
            for oc in range(NCC):
                for th in range(2):
                    pq = pp.tile([128, 512], f32, tag="pp", name="pq")
                    for cc in range(NCC):
                        nc.tensor.matmul(
                            pq,
                            wqt[:, cc, oc * 128 : (oc + 1) * 128],
                            xt[:, cc, th * 512 : (th + 1) * 512],
                            start=(cc == 0),
                            stop=(cc == NCC - 1),
                        )
                    nc.vector.tensor_scalar_add(
                        QT[oc][:, th * 512 : (th + 1) * 512],
                        pq,
                        bq_t[:, oc : oc + 1],
                    )

            # ---- K^T projection (i in halves of 288) ----
            for oc in range(NCC):
                for ih in range(2):
                    pk = pp.tile([128, 512], f32, tag="pp", name="pk")
                    for cc in range(NCC):
                        nc.tensor.matmul(
                            pk[:, :288],
                            wkt[:, cc, oc * 128 : (oc + 1) * 128],
                            ect[:, cc, ih * 288 : (ih + 1) * 288],
                            start=(cc == 0),
                            stop=(cc == NCC - 1),
                        )
                    nc.vector.tensor_scalar_add(
                        KT[oc][:, ih * 288 : (ih + 1) * 288],
                        pk[:, :288],
                        bk_t[:, oc : oc + 1],
                    )

            # ---- V projection into [128, H, VW] layout ----
            for och in range(2):
                for ii in range(NIC):
                    pi = I_CH[ii]
                    pv = pp.tile([128, 512], f32, tag="pp", name="pv")
                    for cc in range(NCC):
                        nc.tensor.matmul(
                            pv[:pi],
                            ect[:, cc, ii * 128 : ii * 128 + pi],
                            wvt[:, cc, och * 512 : (och + 1) * 512],
                            start=(cc == 0),
                            stop=False,
                        )
                    nc.tensor.matmul(
                        pv[:pi],
                        ones_r[0:1, :pi],
                        bv_row[0:1, och * 512 : (och + 1) * 512],
                        start=False,
                        stop=True,
                    )
                    dst = V3[ii][:pi, och * 8 : och * 8 + 8, 0:64]
                    nc.vector.tensor_copy(
                        dst, pv[:pi].rearrange("p (h d) -> p h d", d=64)
                    )

        # prefetch Wo during attention
        with tc.tile_pool(name="ph3", bufs=1) as ph3:
            wot = ph3.tile([128, NCC, C], bf16, tag="wot", name="wot")
            nc.sync.dma_start(out=wot, in_=wo_d[:, :, :])

            # ---- attention ----
            def emit_recip(q):
                # tiles q (tch0) and q+4 (tch1): pairs 2q, 2q+1
                for k in (q, q + 4):
                    with nc.allow_low_precision(reason="1/Z in f32r is fine"):
                        nc.vector.reciprocal(Zi[k], Zc[k])

            def emit_norm(p):
                # normalize pair p (heads 2p, 2p+1); rows at 64*(p%2) + {0,32}
                j = p % 2
                for tch in range(2):
                    tsl = slice(tch * 512, (tch + 1) * 512)
                    k = p // 2 + 4 * tch
                    pb = pbp.tile([128, 512], f32, tag="pb", name="pb")
                    nc.tensor.matmul(
                        pb,
                        sel4[:, j, :],
                        Zi[k][:, :],
                        start=True,
                        stop=True,
                    )
                    nc.vector.tensor_mul(YT[p][:, tsl], YTu[p][:, tsl], pb)

            with tc.tile_pool(name="psp", bufs=2, space="PSUM") as psp, \
                 tc.tile_pool(name="pyp", bufs=3, space="PSUM") as pyp, \
                 tc.tile_pool(name="pbp", bufs=1, space="PSUM") as pbp:
                for k in range(8):
                    nc.gpsimd.memset(Zc[k][:, :], 1.0)
                for h in range(H):
                    oc = h // 2
                    hb = (h % 2) * 64
                    # S^T chunks (both t-halves) -> exp -> sbuf bf16
                    es_h = []
                    for ii in range(NIC):
                        pi = I_CH[ii]
                        ps = psp.tile([128, 1024], f32, tag="ps", name="ps")
                        for tch in range(2):
                            nc.tensor.matmul(
                                ps[:pi, tch * 512 : (tch + 1) * 512],
                                KT[oc][hb : hb + 64, ii * 128 : ii * 128 + pi],
                                QT[oc][hb : hb + 64, tch * 512 : (tch + 1) * 512],
                                start=True,
                                stop=True,
                            )
                        e = exps.tile([128, 1024], bf16, tag="es", name="es")
                        nc.scalar.activation(
                            e[:pi],
                            ps[:pi],
                            mybir.ActivationFunctionType.Exp,
                            scale=float(SCALE),
                        )
                        es_h.append(e)
                    # y^T (64 rows) and Z (row 64) via V augmented with ones col
                    for tch in range(2):
                        tsl = slice(tch * 512, (tch + 1) * 512)
                        py = pyp.tile([128, 512], f32, tag="py", name="py")
                        for ii in range(NIC):
                            pi = I_CH[ii]
                            nc.tensor.matmul(
                                py[:65],
                                V3[ii][:pi, h, 0:65],
                                es_h[ii][:pi, tsl],
                                start=(ii == 0),
                                stop=(ii == NIC - 1),
                            )
                        nc.vector.tensor_copy(YTu[oc][hb : hb + 64, tsl], py[0:64])
                        # Z row (partition 64) -> 32-aligned row of collection tile
                        k = h // 4 + 4 * tch
                        row = 64 * ((h // 2) % 2) + 32 * (h % 2)
                        nc.vector.tensor_copy(
                            Zc[k][row : row + 1], py[64:65]
                        )
                    if h % 4 == 3:
                        emit_recip(h // 4)
                    if h in (5, 6, 9, 10, 13, 14):
                        emit_norm({5: 0, 6: 1, 9: 2, 10: 3, 13: 4, 14: 5}[h])
                emit_norm(6)
                emit_norm(7)

            # ---- output projection ----
            with tc.tile_pool(name="pop", bufs=6, space="PSUM") as pop, \
                 tc.tile_pool(name="osb", bufs=3) as osb:
                for tt in range(8):
                    ot = osb.tile([128, C], f32, tag="osb", name="ot")
                    for och in range(2):
                        po = pop.tile([128, 512], f32, tag="po", name="po")
                        for cc in range(NCC):
                            nc.tensor.matmul(
                                po,
                                YT[cc][:, tt * 128 : (tt + 1) * 128],
                                wot[:, cc, och * 512 : (och + 1) * 512],
                                start=(cc == 0),
                                stop=False,
                            )
                        nc.tensor.matmul(
                            po,
                            ones_r[0:1, 0:128],
                            bo_row[0:1, och * 512 : (och + 1) * 512],
                            start=False,
                            stop=True,
                        )
                        nc.scalar.copy(ot[:, och * 512 : (och + 1) * 512], po)
                    nc.sync.dma_start(out=out_d[tt * 128 : (tt + 1) * 128], in_=ot)

    nc.compile()
    return nc


def _get_nc():
    if "nc" not in _CACHE:
        _CACHE["nc"] = _build_nc()
    return _CACHE["nc"]


def _to_chunked_bf16(a):
    # [R, N] f32 (R = 1024 rows of the contraction dim) -> [128, R//128, N] bf16
    r, n = a.shape
    return np.ascontiguousarray(
        a.reshape(r // 128, 128, n).transpose(1, 0, 2)
    ).astype(ml_dtypes.bfloat16)


def _prep_in_maps(x, encoder_output, Wq, bq, Wkv, bkv, Wo, bo):
    f = np.float32
    bf = ml_dtypes.bfloat16
    x = np.asarray(x, f)
    enc = np.asarray(encoder_output, f)
    Wq = np.asarray(Wq, f)
    wkv = np.asarray(Wkv, f)
    Wo = np.asarray(Wo, f)
    bq = np.asarray(bq, f)
    bkv = np.asarray(bkv, f)
    bo = np.asarray(bo, f)
    shared = {
        "wq": _to_chunked_bf16(np.ascontiguousarray(Wq.T)),
        "wk": _to_chunked_bf16(np.ascontiguousarray(wkv[:C].T)),
        "wv": _to_chunked_bf16(np.ascontiguousarray(wkv[C:].T)),
        "wo": _to_chunked_bf16(np.ascontiguousarray(Wo.T)),
        "bqp": np.ascontiguousarray(bq.reshape(NCC, 128).T),
        "bkp": np.ascontiguousarray(bkv[:C].reshape(NCC, 128).T),
        "bv": np.ascontiguousarray(bkv[C:]).astype(bf),
        "bo": np.ascontiguousarray(bo).astype(bf),
    }
    return [
        dict(
            shared,
            xt=_to_chunked_bf16(np.ascontiguousarray(x[b].T)),
            ect=_to_chunked_bf16(np.ascontiguousarray(enc[b].T)),
        )
        for b in range(x.shape[0])
    ]


def kernel(x, encoder_output, Wq, bq, Wkv, bkv, Wo, bo):
    from concourse.bass_utils import run_bass_kernel_spmd

    nc = _get_nc()
    in_maps = _prep_in_maps(x, encoder_output, Wq, bq, Wkv, bkv, Wo, bo)
    res = run_bass_kernel_spmd(nc, in_maps, list(range(len(in_maps)))).results
    return np.stack([res[b]["out"] for b in range(len(res))]).astype(np.float32)
